# revision 17
# baseline (speedup 1.0000x reference)
"""ChebyshevKANLayer on 8 Trainium2 NeuronCores.

y = silu(x) @ Wb + sum_d (x * T_d(xs)) @ Wc[:, :, d]
  xs = per-row rescale of x to [-1, 1]; T_d = Chebyshev polynomials.

Sharding: data-parallel over the batch dim (4096 -> 8 x 512 rows),
weights replicated; no collectives, host concatenates the shards.

All matmul operands are fp16 (1 PE cycle/row, same rate as bf16 but
8x finer mantissa: end-to-end rel err ~4e-3 vs 2e-2 budget). The
Chebyshev recurrence runs on DVE in fp16 storage / fp32 ALU, which
enables the DVE 2x_1p mode (327ns per [128,512] op) and kills all
operand-cast traffic. Host packs weights as [wb | wc_d0 | .. | wc_d7]
so each contraction tile streams with two DMAs (wA: silu+d0 columns,
wB: d1..d7 columns).

Schedule (per core, cost-model-driven):
  - ~25 tiny dummy matmuls warm the PE P-state while the first DMAs
    land (PE ramps to 2.4GHz after 3us of continuous execution).
  - phase 1 (stats-independent): per k, d0 matmuls (lhsT = raw x f16)
    and silu matmuls (sigmoid on ACT, multiply on DVE). Meanwhile row
    min/max stats run on DVE+gpsimd from the natural-layout copy,
    tiny strided DMAs gather per-row scale/offset into [1,512] rows,
    and gpsimd.partition_broadcast forms the [128,512] u-coefficient
    tiles -- no PE, no PSUM involvement.
  - phase 2: per k, the fp16 G-chain (G_d = u*G_{d-1} - G_{d-2}) on
    DVE, then 56 accumulating matmuls; weight DMA (5.1us) and chain
    (5.2us) both fit inside the 11.9us PE window, pipelined one k
    ahead.
  - epilogue: the last k's matmuls run bank-major so each PSUM bank
    stops early and drains (ACT copy to f16 + DMA) under the
    remaining matmuls.
"""

import numpy as np

from concourse import bacc, mybir, tile
from concourse.bass_utils import run_bass_kernel_spmd

B, IN, OUT, DEG = 4096, 1024, 1024, 8
NCORES = 8
BS = B // NCORES  # 512 rows per core
KT = IN // 128  # 8 contraction tiles
NB = BS // 128  # 4 batch tiles per core
NO = OUT // 512  # 2 output column tiles
NDUMMY = 20

F32 = mybir.dt.float32
F16 = mybir.dt.float16
BF16 = mybir.dt.bfloat16
ALU = mybir.AluOpType
AF = mybir.ActivationFunctionType
AX = mybir.AxisListType


CHAIN_F16 = True   # f16 chain storage (no casts) vs fp32 chain + ACT casts
MM_BF16 = False    # bf16 matmul operands/weights vs f16


def _build_kernel(tc, out, xt, xn, wf, repeat=1, chain_f16=True, mm_bf16=False, split_wdma=False, resident=True):
    nc = tc.nc
    MD = BF16 if mm_bf16 else F16   # matmul operand dtype
    CD = F16 if chain_f16 else F32  # chain storage dtype
    from contextlib import ExitStack

    octx = ExitStack()
    cpool = octx.enter_context(tc.tile_pool(name="const", bufs=1))
    zz = cpool.tile([128, 128], MD)
    hb = cpool.tile([128, BS], CD)  # 0.5 broadcast (for G1 = (x/2)*u)
    sb = cpool.tile([128, BS], CD)  # per-column s   (u = x*s + t)
    tb = cpool.tile([128, BS], CD)  # per-column t
    s_row = cpool.tile([1, BS], CD)
    t_row = cpool.tile([1, BS], CD)

    with (
        tc.tile_pool(name="psum_acc", bufs=1, space="PSUM") as pacc,
        tc.tile_pool(name="wa", bufs=1) as wapool,
        tc.tile_pool(name="wb", bufs=1 if resident else 3) as wbpool,
        tc.tile_pool(name="sg", bufs=2) as sgpool,
        tc.tile_pool(name="xta", bufs=1) as xtpool,
        tc.tile_pool(name="xna", bufs=1) as xnpool,
        tc.tile_pool(name="g", bufs=2) as gpool,
        tc.tile_pool(name="gb", bufs=2) as gbpool,
        tc.tile_pool(name="u", bufs=2) as upool,
        tc.tile_pool(name="silu", bufs=1) as slpool,
        tc.tile_pool(name="o", bufs=2) as opool,
        tc.tile_pool(name="stats", bufs=1) as spool,
    ):
        po = [
            [
                pacc.tile([128, 512], F32, tag=f"po{t}{j}", name=f"po{t}{j}")
                for j in range(NO)
            ]
            for t in range(NB)
        ]
        xta = xtpool.tile([128, KT * BS], F16, tag="xta", name="xta")
        xna = xnpool.tile([128, NB * IN], F16, tag="xna", name="xna")

        def xk(k):  # [128, BS] fp16 block of x^T for contraction tile k
            return xta[:, k * BS : (k + 1) * BS]

        # weight tiles: resident mode keeps all 18MB of packed weights in
        # SBUF (144KB/partition) -- DMA'd once, reused by every rep, so the
        # steady-state rep moves only the 1MB output
        was = [
            wapool.tile([128, 2 * OUT], MD, tag=f"wa{k}", name=f"wa{k}")
            for k in range(KT)
        ]
        wbs = (
            [
                wbpool.tile(
                    [128, (DEG - 1) * OUT], MD, tag=f"wbt{k}", name=f"wbt{k}"
                )
                for k in range(KT)
            ]
            if resident
            else None
        )

        def dma_wa(k):
            if split_wdma:
                for c in range(2):
                    nc.sync.dma_start(
                        out=was[k][:, c * OUT : (c + 1) * OUT],
                        in_=wf[k * 128 : (k + 1) * 128, c * OUT : (c + 1) * OUT],
                    )
            else:
                nc.sync.dma_start(
                    out=was[k][:], in_=wf[k * 128 : (k + 1) * 128, 0 : 2 * OUT]
                )

        def dma_xn(t):
            nc.sync.dma_start(
                out=xna[:, t * IN : (t + 1) * IN],
                in_=xn[:, t * IN : (t + 1) * IN],
            )

        for rep in range(repeat):
            first = rep == 0
            if first:
                # tiny self-contained matmuls keep the PE busy (P-state
                # ramp) while the first input DMAs land
                nc.vector.memset(zz[:], 0.0)
                nc.vector.memset(hb[:], 0.5)
                for _ in range(NDUMMY):
                    nc.tensor.matmul(
                        po[0][0][:, 0:128], lhsT=zz[:], rhs=zz[:],
                        start=True, stop=True,
                    )

            # ---- input + phase-1 weight DMA stream (SP queue) ----
            # order tuned so the PE never starves: x block k0, wa0, x rest
            # (split), then wa_k interleaved with the stats loads
            if first:
                nc.sync.dma_start(out=xta[:, 0:BS], in_=xt[:, 0:BS])
                dma_wa(0)
                nc.sync.dma_start(out=xta[:, BS : 4 * BS], in_=xt[:, BS : 4 * BS])
                dma_wa(1)
                nc.sync.dma_start(out=xta[:, 4 * BS :], in_=xt[:, 4 * BS :])
                dma_xn(0)
                dma_wa(2)
                dma_xn(1)
                dma_wa(3)
                dma_xn(2)
                dma_wa(4)
                dma_xn(3)
                for k in range(5, KT):
                    dma_wa(k)
            elif not resident:
                for k in range(KT):
                    dma_wa(k)

            # ---- row stats -> sb/tb broadcast tiles (no PE, no PSUM) ----
            if first:
                for t in range(NB):
                    xnt = xna[:, t * IN : (t + 1) * IN]
                    mx = spool.tile([128, 1], F32, tag=f"mx{t}", name="mx")
                    mn = spool.tile([128, 1], F32, tag=f"mn{t}", name="mn")
                    nc.vector.tensor_reduce(mx[:], xnt, axis=AX.X, op=ALU.max)
                    nc.vector.tensor_reduce(mn[:], xnt, axis=AX.X, op=ALU.min)
                    d = spool.tile([128, 1], F32, tag=f"d{t}", name="d")
                    nc.vector.tensor_tensor(d[:], mx[:], mn[:], ALU.subtract)
                    r = spool.tile([128, 1], F32, tag=f"r{t}", name="r")
                    nc.vector.reciprocal(r[:], d[:])
                    sc = spool.tile([128, 1], CD, tag=f"sc{t}", name="sc")
                    nc.vector.tensor_scalar(sc[:], r[:], 4.0, None, ALU.mult)
                    tmp = spool.tile([128, 1], F32, tag=f"tm{t}", name="tm")
                    nc.vector.tensor_tensor(tmp[:], mn[:], sc[:], ALU.mult)
                    tcn = spool.tile([128, 1], CD, tag=f"tc{t}", name="tc")
                    nc.vector.tensor_scalar(
                        tcn[:], tmp[:], -1.0, -2.0, ALU.mult, ALU.add
                    )
                    tsl = slice(t * 128, (t + 1) * 128)
                    # strided SBUF->SBUF gathers: [128,1] column -> row slice
                    nc.scalar.dma_start(out=s_row[0:1, tsl], in_=sc[:, 0:1])
                    nc.scalar.dma_start(out=t_row[0:1, tsl], in_=tcn[:, 0:1])
                nc.gpsimd.partition_broadcast(sb[:], s_row[0:1, :])
                nc.gpsimd.partition_broadcast(tb[:], t_row[0:1, :])

            # ---- silu = x * sigmoid(x): sigmoid on ACT, multiply on DVE ----
            sls = []
            xbs = []
            for k in range(KT):
                sg = sgpool.tile([128, BS], MD, tag="sg", name="sg")
                sl = slpool.tile([128, BS], MD, tag=f"sl{k}", name=f"sl{k}")
                sls.append(sl)
                nc.scalar.activation(sg[:], xk(k), AF.Sigmoid)
                nc.vector.tensor_tensor(sl[:], sg[:], xk(k), ALU.mult)
                if mm_bf16:
                    xb = slpool.tile([128, BS], MD, tag=f"xb{k}", name=f"xb{k}")
                    nc.scalar.activation(xb[:], xk(k), AF.Copy)
                    xbs.append(xb)
                else:
                    xbs.append(None)

            # ---- phase 1: d0 + silu matmuls (stats-independent) ----
            for k in range(KT):
                wa = was[k]
                for t in range(NB):
                    for j in range(NO):
                        xlhs = xbs[k] if mm_bf16 else xk(k)
                        nc.tensor.matmul(
                            po[t][j][:],
                            lhsT=xlhs[:, t * 128 : (t + 1) * 128],
                            rhs=wa[:, OUT + j * 512 : OUT + (j + 1) * 512],
                            start=(k == 0),
                            stop=False,
                        )
                for t in range(NB):
                    for j in range(NO):
                        nc.tensor.matmul(
                            po[t][j][:],
                            lhsT=sls[k][:, t * 128 : (t + 1) * 128],
                            rhs=wa[:, j * 512 : (j + 1) * 512],
                            start=False,
                            stop=False,
                        )

            # ---- phase 2: chebyshev chain + d1..7 matmuls ----
            for k in range(KT):
                if resident:
                    wb = wbs[k]
                else:
                    wb = wbpool.tile(
                        [128, (DEG - 1) * OUT], MD, tag="wbt", name="wbt"
                    )
                if (not resident or first) and split_wdma:
                    for c in range(DEG - 1):
                        nc.sync.dma_start(
                            out=wb[:, c * OUT : (c + 1) * OUT],
                            in_=wf[k * 128 : (k + 1) * 128,
                                   (2 + c) * OUT : (3 + c) * OUT],
                        )
                elif not resident or first:
                    nc.sync.dma_start(
                        out=wb[:],
                        in_=wf[k * 128 : (k + 1) * 128, 2 * OUT : (DEG + 1) * OUT],
                    )
                gall = gpool.tile([128, (DEG - 1) * BS], CD, tag="gall", name="gall")

                def G(i):  # chain slots G_1..G_7
                    return gall[:, (i - 1) * BS : i * BS]

                if chain_f16 and not mm_bf16:
                    Gmm = G  # chain tiles feed the PE directly
                else:
                    gba = gbpool.tile(
                        [128, (DEG - 1) * BS], MD, tag="gba", name="gba"
                    )

                    def Gmm(i):
                        return gba[:, (i - 1) * BS : i * BS]

                ut = upool.tile([128, BS], CD, tag="ut", name="ut")
                xh = upool.tile([128, BS], CD, tag="xh", name="xh")
                nc.vector.tensor_tensor(ut[:], xk(k), sb[:], ALU.mult)
                nc.vector.tensor_tensor(ut[:], ut[:], tb[:], ALU.add)
                nc.vector.tensor_tensor(xh[:], xk(k), hb[:], ALU.mult)
                nc.vector.tensor_tensor(G(1), xh[:], ut[:], ALU.mult)
                if Gmm is not G:
                    nc.scalar.activation(Gmm(1), G(1), AF.Copy)
                for dg in range(2, DEG):
                    tmpd = upool.tile([128, BS], CD, tag="td", name="td")
                    nc.vector.tensor_tensor(tmpd[:], ut[:], G(dg - 1), ALU.mult)
                    prev2 = xk(k) if dg == 2 else G(dg - 2)
                    nc.vector.tensor_tensor(G(dg), tmpd[:], prev2, ALU.subtract)
                    if Gmm is not G:
                        nc.scalar.activation(Gmm(dg), G(dg), AF.Copy)

                last = k == KT - 1
                if not last:
                    for m in range(1, DEG):
                        for t in range(NB):
                            for j in range(NO):
                                nc.tensor.matmul(
                                    po[t][j][:],
                                    lhsT=Gmm(m)[:, t * 128 : (t + 1) * 128],
                                    rhs=wb[:, (m - 1) * OUT + j * 512 :
                                           (m - 1) * OUT + (j + 1) * 512],
                                    start=False,
                                    stop=False,
                                )
                else:
                    # bank-major: stop + drain each PSUM bank under the
                    # remaining matmuls; alternate ACT/DVE so the drains
                    # don't serialize on one queue
                    for bank, (t, j) in enumerate(
                        (t, j) for t in range(NB) for j in range(NO)
                    ):
                        for m in range(1, DEG):
                            nc.tensor.matmul(
                                po[t][j][:],
                                lhsT=Gmm(m)[:, t * 128 : (t + 1) * 128],
                                rhs=wb[:, (m - 1) * OUT + j * 512 :
                                       (m - 1) * OUT + (j + 1) * 512],
                                start=False,
                                stop=(m == DEG - 1),
                            )
                        ot = opool.tile(
                            [128, 512], F16, tag=f"ot{bank % 2}", name="ot"
                        )
                        if bank % 2 == 0:
                            nc.scalar.activation(ot[:], po[t][j][:], AF.Copy)
                            dma_eng = nc.scalar
                        else:
                            nc.vector.tensor_copy(ot[:], po[t][j][:])
                            dma_eng = nc.sync
                        dma_eng.dma_start(
                            out=out[t * 128 : (t + 1) * 128,
                                    j * 512 : (j + 1) * 512],
                            in_=ot[:],
                        )
    octx.close()


_NC_CACHE = {}


def build_nc(repeat=1, chain_f16=None, mm_bf16=None, split_wdma=False, resident=None):
    if chain_f16 is None:
        chain_f16 = CHAIN_F16
    if mm_bf16 is None:
        mm_bf16 = MM_BF16
    if resident is None:
        # resident weights measured slower on HW (SBUF ~98% full appears to
        # cost more in PE read-port contention than the weight re-DMA saves)
        resident = False
    key = (repeat, chain_f16, mm_bf16, split_wdma, resident)
    if key in _NC_CACHE:
        return _NC_CACHE[key]
    nc = bacc.Bacc(
        "TRN2", target_bir_lowering=False, debug=False, num_devices=NCORES
    )
    WD = BF16 if mm_bf16 else F16
    xt = nc.dram_tensor("xt", [128, KT * BS], F16, kind="ExternalInput").ap()
    xn = nc.dram_tensor("xn", [128, NB * IN], F16, kind="ExternalInput").ap()
    wf = nc.dram_tensor("wf", [IN, (DEG + 1) * OUT], WD, kind="ExternalInput").ap()
    out = nc.dram_tensor("out", [BS, OUT], F16, kind="ExternalOutput").ap()
    with tile.TileContext(nc) as tc:
        _build_kernel(
            tc, out, xt, xn, wf, repeat=repeat,
            chain_f16=chain_f16, mm_bf16=mm_bf16, split_wdma=split_wdma,
            resident=resident,
        )
    nc.compile()
    _NC_CACHE[key] = nc
    return nc


def make_in_maps(x, base_weight, cheb_weight):
    x = np.asarray(x, dtype=np.float32)
    wb = np.asarray(base_weight, dtype=np.float32)
    wc = np.asarray(cheb_weight, dtype=np.float32)
    # [wb | wc_d0 | .. | wc_d7] -> [IN, 9*OUT] fp16
    if MM_BF16:
        import ml_dtypes
        wdt = ml_dtypes.bfloat16
    else:
        wdt = np.float16
    wf = np.concatenate(
        [wb[:, None, :], wc.transpose(0, 2, 1)], axis=1
    ).reshape(IN, (DEG + 1) * OUT).astype(wdt)
    wf = np.ascontiguousarray(wf)
    in_maps = []
    for c in range(NCORES):
        shard = x[c * BS : (c + 1) * BS].astype(np.float16)  # [BS, IN]
        # xt: [128, KT*BS], block k = x^T rows k*128:(k+1)*128
        xt = np.ascontiguousarray(
            shard.T.reshape(KT, 128, BS).transpose(1, 0, 2).reshape(128, KT * BS)
        )
        # xn: [128, NB*IN], block t = rows t*128:(t+1)*128 of the shard
        xn = np.ascontiguousarray(
            shard.reshape(NB, 128, IN).transpose(1, 0, 2).reshape(128, NB * IN)
        )
        in_maps.append({"xt": xt, "xn": xn, "wf": wf})
    return in_maps


def kernel(x, base_weight, cheb_weight, degree=DEG, **_):
    assert int(degree) == DEG
    nc = build_nc()
    in_maps = make_in_maps(x, base_weight, cheb_weight)
    res = run_bass_kernel_spmd(nc, in_maps, list(range(NCORES)))
    return np.concatenate(
        [r["out"].astype(np.float32) for r in res.results], axis=0
    )


# revision 21
# speedup vs baseline: 1.1343x; 1.1343x over previous
"""ChebyshevKANLayer on 8 Trainium2 NeuronCores.

y = silu(x) @ Wb + sum_d (x * T_d(xs)) @ Wc[:, :, d]
  xs = per-row rescale of x to [-1, 1]; T_d = Chebyshev polynomials.

Sharding: data-parallel over the batch dim (4096 -> 8 x 512 rows),
weights replicated; no collectives, host concatenates the shards.

All matmul operands are fp16 (1 PE cycle/row, same rate as bf16 but
8x finer mantissa: end-to-end rel err ~4e-3 vs 2e-2 budget). The
Chebyshev recurrence runs on DVE in fp16 storage / fp32 ALU, which
enables the DVE 2x_1p mode (327ns per [128,512] op) and kills all
operand-cast traffic. Host packs weights as [wb | wc_d0 | .. | wc_d7]
so each contraction tile streams with two DMAs (wA: silu+d0 columns,
wB: d1..d7 columns).

Schedule (per core, cost-model-driven):
  - ~25 tiny dummy matmuls warm the PE P-state while the first DMAs
    land (PE ramps to 2.4GHz after 3us of continuous execution).
  - phase 1 (stats-independent): per k, d0 matmuls (lhsT = raw x f16)
    and silu matmuls (sigmoid on ACT, multiply on DVE). Meanwhile row
    min/max stats run on DVE+gpsimd from the natural-layout copy,
    tiny strided DMAs gather per-row scale/offset into [1,512] rows,
    and gpsimd.partition_broadcast forms the [128,512] u-coefficient
    tiles -- no PE, no PSUM involvement.
  - phase 2: per k, the fp16 G-chain (G_d = u*G_{d-1} - G_{d-2}) on
    DVE, then 56 accumulating matmuls; weight DMA (5.1us) and chain
    (5.2us) both fit inside the 11.9us PE window, pipelined one k
    ahead.
  - epilogue: the last k's matmuls run bank-major so each PSUM bank
    stops early and drains (ACT copy to f16 + DMA) under the
    remaining matmuls.
"""

import numpy as np

from concourse import bacc, mybir, tile
from concourse.bass_utils import run_bass_kernel_spmd

B, IN, OUT, DEG = 4096, 1024, 1024, 8
NCORES = 8
BS = B // NCORES  # 512 rows per core
KT = IN // 128  # 8 contraction tiles
NB = BS // 128  # 4 batch tiles per core
NO = OUT // 512  # 2 output column tiles
NDUMMY = 20

F32 = mybir.dt.float32
F16 = mybir.dt.float16
BF16 = mybir.dt.bfloat16
ALU = mybir.AluOpType
AF = mybir.ActivationFunctionType
AX = mybir.AxisListType


CHAIN_F16 = True   # f16 chain storage (no casts) vs fp32 chain + ACT casts
MM_BF16 = False    # bf16 matmul operands/weights vs f16


def _build_kernel(tc, out, xt, xn, wf, repeat=1, chain_f16=True, mm_bf16=False, split_wdma=False, resident=True):
    nc = tc.nc
    MD = BF16 if mm_bf16 else F16   # matmul operand dtype
    CD = F16 if chain_f16 else F32  # chain storage dtype
    from contextlib import ExitStack

    octx = ExitStack()
    cpool = octx.enter_context(tc.tile_pool(name="const", bufs=1))
    zz = cpool.tile([128, 128], MD)
    hb = cpool.tile([128, BS], CD)  # 0.5 broadcast (for G1 = (x/2)*u)
    sb = cpool.tile([128, BS], CD)  # per-column s   (u = x*s + t)
    tb = cpool.tile([128, BS], CD)  # per-column t
    s_row = cpool.tile([1, BS], CD)
    t_row = cpool.tile([1, BS], CD)

    with (
        tc.tile_pool(name="psum_acc", bufs=1, space="PSUM") as pacc,
        tc.tile_pool(name="wa", bufs=1) as wapool,
        tc.tile_pool(name="wb", bufs=1 if resident else 3) as wbpool,
        tc.tile_pool(name="sg", bufs=2) as sgpool,
        tc.tile_pool(name="xta", bufs=1) as xtpool,
        tc.tile_pool(name="xna", bufs=1) as xnpool,
        tc.tile_pool(name="g", bufs=2) as gpool,
        tc.tile_pool(name="gb", bufs=2) as gbpool,
        tc.tile_pool(name="u", bufs=2) as upool,
        tc.tile_pool(name="silu", bufs=1) as slpool,
        tc.tile_pool(name="o", bufs=2) as opool,
        tc.tile_pool(name="stats", bufs=1) as spool,
    ):
        po = [
            [
                pacc.tile([128, 512], F32, tag=f"po{t}{j}", name=f"po{t}{j}")
                for j in range(NO)
            ]
            for t in range(NB)
        ]
        xta = xtpool.tile([128, KT * BS], F16, tag="xta", name="xta")
        xna = xnpool.tile([128, NB * IN], F16, tag="xna", name="xna")

        def xk(k):  # [128, BS] fp16 block of x^T for contraction tile k
            return xta[:, k * BS : (k + 1) * BS]

        # weight tiles: resident mode keeps all 18MB of packed weights in
        # SBUF (144KB/partition) -- DMA'd once, reused by every rep, so the
        # steady-state rep moves only the 1MB output
        was = [
            wapool.tile([128, 2 * OUT], MD, tag=f"wa{k}", name=f"wa{k}")
            for k in range(KT)
        ]
        wbs = (
            [
                wbpool.tile(
                    [128, (DEG - 1) * OUT], MD, tag=f"wbt{k}", name=f"wbt{k}"
                )
                for k in range(KT)
            ]
            if resident
            else None
        )

        def dma_wa(k):
            if split_wdma:
                for c in range(2):
                    nc.sync.dma_start(
                        out=was[k][:, c * OUT : (c + 1) * OUT],
                        in_=wf[k * 128 : (k + 1) * 128, c * OUT : (c + 1) * OUT],
                    )
            else:
                nc.sync.dma_start(
                    out=was[k][:], in_=wf[k * 128 : (k + 1) * 128, 0 : 2 * OUT]
                )

        def dma_xn(t):
            nc.sync.dma_start(
                out=xna[:, t * IN : (t + 1) * IN],
                in_=xn[:, t * IN : (t + 1) * IN],
            )

        for rep in range(repeat):
            first = rep == 0
            if first:
                # tiny self-contained matmuls keep the PE busy (P-state
                # ramp) while the first input DMAs land
                nc.vector.memset(zz[:], 0.0)
                nc.vector.memset(hb[:], 0.5)
                for _ in range(NDUMMY):
                    nc.tensor.matmul(
                        po[0][0][:, 0:128], lhsT=zz[:], rhs=zz[:],
                        start=True, stop=True,
                    )

            # ---- input + phase-1 weight DMA stream (SP queue) ----
            # order tuned so the PE never starves: x block k0, wa0, x rest
            # (split), then wa_k interleaved with the stats loads
            if first:
                nc.sync.dma_start(out=xta[:, 0:BS], in_=xt[:, 0:BS])
                dma_wa(0)
                nc.sync.dma_start(out=xta[:, BS : 4 * BS], in_=xt[:, BS : 4 * BS])
                dma_wa(1)
                nc.sync.dma_start(out=xta[:, 4 * BS :], in_=xt[:, 4 * BS :])
                dma_xn(0)
                dma_wa(2)
                dma_xn(1)
                dma_wa(3)
                dma_xn(2)
                dma_wa(4)
                dma_xn(3)
                for k in range(5, KT):
                    dma_wa(k)
            elif not resident:
                for k in range(KT):
                    dma_wa(k)

            # ---- row stats -> sb/tb broadcast tiles (no PE, no PSUM) ----
            if first:
                for t in range(NB):
                    xnt = xna[:, t * IN : (t + 1) * IN]
                    mx = spool.tile([128, 1], F32, tag=f"mx{t}", name="mx")
                    mn = spool.tile([128, 1], F32, tag=f"mn{t}", name="mn")
                    nc.vector.tensor_reduce(mx[:], xnt, axis=AX.X, op=ALU.max)
                    nc.vector.tensor_reduce(mn[:], xnt, axis=AX.X, op=ALU.min)
                    d = spool.tile([128, 1], F32, tag=f"d{t}", name="d")
                    nc.vector.tensor_tensor(d[:], mx[:], mn[:], ALU.subtract)
                    r = spool.tile([128, 1], F32, tag=f"r{t}", name="r")
                    nc.vector.reciprocal(r[:], d[:])
                    sc = spool.tile([128, 1], CD, tag=f"sc{t}", name="sc")
                    nc.vector.tensor_scalar(sc[:], r[:], 4.0, None, ALU.mult)
                    tmp = spool.tile([128, 1], F32, tag=f"tm{t}", name="tm")
                    nc.vector.tensor_tensor(tmp[:], mn[:], sc[:], ALU.mult)
                    tcn = spool.tile([128, 1], CD, tag=f"tc{t}", name="tc")
                    nc.vector.tensor_scalar(
                        tcn[:], tmp[:], -1.0, -2.0, ALU.mult, ALU.add
                    )
                    tsl = slice(t * 128, (t + 1) * 128)
                    # strided SBUF->SBUF gathers: [128,1] column -> row slice
                    nc.scalar.dma_start(out=s_row[0:1, tsl], in_=sc[:, 0:1])
                    nc.scalar.dma_start(out=t_row[0:1, tsl], in_=tcn[:, 0:1])
                nc.gpsimd.partition_broadcast(sb[:], s_row[0:1, :])
                nc.gpsimd.partition_broadcast(tb[:], t_row[0:1, :])

            # ---- silu = x * sigmoid(x): sigmoid on ACT, multiply on DVE ----
            sls = []
            xbs = []
            for k in range(KT):
                sg = sgpool.tile([128, BS], MD, tag="sg", name="sg")
                sl = slpool.tile([128, BS], MD, tag=f"sl{k}", name=f"sl{k}")
                sls.append(sl)
                nc.scalar.activation(sg[:], xk(k), AF.Sigmoid)
                nc.vector.tensor_tensor(sl[:], sg[:], xk(k), ALU.mult)
                if mm_bf16:
                    xb = slpool.tile([128, BS], MD, tag=f"xb{k}", name=f"xb{k}")
                    nc.scalar.activation(xb[:], xk(k), AF.Copy)
                    xbs.append(xb)
                else:
                    xbs.append(None)

            # ---- phase 1: d0 + silu matmuls (stats-independent) ----
            for k in range(KT):
                wa = was[k]
                for t in range(NB):
                    for j in range(NO):
                        xlhs = xbs[k] if mm_bf16 else xk(k)
                        nc.tensor.matmul(
                            po[t][j][:],
                            lhsT=xlhs[:, t * 128 : (t + 1) * 128],
                            rhs=wa[:, OUT + j * 512 : OUT + (j + 1) * 512],
                            start=(k == 0),
                            stop=False,
                        )
                for t in range(NB):
                    for j in range(NO):
                        nc.tensor.matmul(
                            po[t][j][:],
                            lhsT=sls[k][:, t * 128 : (t + 1) * 128],
                            rhs=wa[:, j * 512 : (j + 1) * 512],
                            start=False,
                            stop=False,
                        )

            # ---- phase 2: chebyshev chain + d1..7 matmuls ----
            for k in range(KT):
                if resident:
                    wb = wbs[k]
                else:
                    wb = wbpool.tile(
                        [128, (DEG - 1) * OUT], MD, tag="wbt", name="wbt"
                    )
                if (not resident or first) and split_wdma:
                    for c in range(DEG - 1):
                        nc.sync.dma_start(
                            out=wb[:, c * OUT : (c + 1) * OUT],
                            in_=wf[k * 128 : (k + 1) * 128,
                                   (2 + c) * OUT : (3 + c) * OUT],
                        )
                elif not resident or first:
                    nc.sync.dma_start(
                        out=wb[:],
                        in_=wf[k * 128 : (k + 1) * 128, 2 * OUT : (DEG + 1) * OUT],
                    )
                gall = gpool.tile([128, (DEG - 1) * BS], CD, tag="gall", name="gall")

                def G(i):  # chain slots G_1..G_7
                    return gall[:, (i - 1) * BS : i * BS]

                if chain_f16 and not mm_bf16:
                    Gmm = G  # chain tiles feed the PE directly
                else:
                    gba = gbpool.tile(
                        [128, (DEG - 1) * BS], MD, tag="gba", name="gba"
                    )

                    def Gmm(i):
                        return gba[:, (i - 1) * BS : i * BS]

                ut = upool.tile([128, BS], CD, tag="ut", name="ut")
                xh = upool.tile([128, BS], CD, tag="xh", name="xh")
                nc.vector.tensor_tensor(ut[:], xk(k), sb[:], ALU.mult)
                nc.vector.tensor_tensor(ut[:], ut[:], tb[:], ALU.add)
                nc.vector.tensor_tensor(xh[:], xk(k), hb[:], ALU.mult)
                nc.vector.tensor_tensor(G(1), xh[:], ut[:], ALU.mult)
                if Gmm is not G:
                    nc.scalar.activation(Gmm(1), G(1), AF.Copy)
                for dg in range(2, DEG):
                    tmpd = upool.tile([128, BS], CD, tag="td", name="td")
                    nc.vector.tensor_tensor(tmpd[:], ut[:], G(dg - 1), ALU.mult)
                    prev2 = xk(k) if dg == 2 else G(dg - 2)
                    nc.vector.tensor_tensor(G(dg), tmpd[:], prev2, ALU.subtract)
                    if Gmm is not G:
                        nc.scalar.activation(Gmm(dg), G(dg), AF.Copy)

                last = k == KT - 1
                if not last:
                    for m in range(1, DEG):
                        for t in range(NB):
                            for j in range(NO):
                                nc.tensor.matmul(
                                    po[t][j][:],
                                    lhsT=Gmm(m)[:, t * 128 : (t + 1) * 128],
                                    rhs=wb[:, (m - 1) * OUT + j * 512 :
                                           (m - 1) * OUT + (j + 1) * 512],
                                    start=False,
                                    stop=False,
                                )
                else:
                    # bank-major: stop + drain each PSUM bank under the
                    # remaining matmuls; alternate ACT/DVE so the drains
                    # don't serialize on one queue
                    for bank, (t, j) in enumerate(
                        (t, j) for t in range(NB) for j in range(NO)
                    ):
                        for m in range(1, DEG):
                            nc.tensor.matmul(
                                po[t][j][:],
                                lhsT=Gmm(m)[:, t * 128 : (t + 1) * 128],
                                rhs=wb[:, (m - 1) * OUT + j * 512 :
                                       (m - 1) * OUT + (j + 1) * 512],
                                start=False,
                                stop=(m == DEG - 1),
                            )
                        ot = opool.tile(
                            [128, 512], F16, tag=f"ot{bank % 2}", name="ot"
                        )
                        if bank % 2 == 0:
                            nc.scalar.activation(ot[:], po[t][j][:], AF.Copy)
                            dma_eng = nc.scalar
                        else:
                            nc.vector.tensor_copy(ot[:], po[t][j][:])
                            dma_eng = nc.sync
                        dma_eng.dma_start(
                            out=out[t * 128 : (t + 1) * 128,
                                    j * 512 : (j + 1) * 512],
                            in_=ot[:],
                        )
    octx.close()


_NC_CACHE = {}


def build_nc(repeat=1, chain_f16=None, mm_bf16=None, split_wdma=False, resident=None):
    if chain_f16 is None:
        chain_f16 = CHAIN_F16
    if mm_bf16 is None:
        mm_bf16 = MM_BF16
    if resident is None:
        # resident weights measured slower on HW (SBUF ~98% full appears to
        # cost more in PE read-port contention than the weight re-DMA saves)
        resident = False
    key = (repeat, chain_f16, mm_bf16, split_wdma, resident)
    if key in _NC_CACHE:
        return _NC_CACHE[key]
    nc = bacc.Bacc(
        "TRN2", target_bir_lowering=False, debug=False, num_devices=NCORES
    )
    WD = BF16 if mm_bf16 else F16
    xt = nc.dram_tensor("xt", [128, KT * BS], F16, kind="ExternalInput").ap()
    xn = nc.dram_tensor("xn", [128, NB * IN], F16, kind="ExternalInput").ap()
    wf = nc.dram_tensor("wf", [IN, (DEG + 1) * OUT], WD, kind="ExternalInput").ap()
    out = nc.dram_tensor("out", [BS, OUT], F16, kind="ExternalOutput").ap()
    with tile.TileContext(nc) as tc:
        _build_kernel(
            tc, out, xt, xn, wf, repeat=repeat,
            chain_f16=chain_f16, mm_bf16=mm_bf16, split_wdma=split_wdma,
            resident=resident,
        )
    nc.compile()
    _NC_CACHE[key] = nc
    return nc


def make_in_maps(x, base_weight, cheb_weight):
    x = np.asarray(x, dtype=np.float32)
    wb = np.asarray(base_weight, dtype=np.float32)
    wc = np.asarray(cheb_weight, dtype=np.float32)
    # [wb | wc_d0 | .. | wc_d7] -> [IN, 9*OUT] fp16
    if MM_BF16:
        import ml_dtypes
        wdt = ml_dtypes.bfloat16
    else:
        wdt = np.float16
    wf = np.concatenate(
        [wb[:, None, :], wc.transpose(0, 2, 1)], axis=1
    ).reshape(IN, (DEG + 1) * OUT).astype(wdt)
    wf = np.ascontiguousarray(wf)
    in_maps = []
    for c in range(NCORES):
        shard = x[c * BS : (c + 1) * BS].astype(np.float16)  # [BS, IN]
        # xt: [128, KT*BS], block k = x^T rows k*128:(k+1)*128
        xt = np.ascontiguousarray(
            shard.T.reshape(KT, 128, BS).transpose(1, 0, 2).reshape(128, KT * BS)
        )
        # xn: [128, NB*IN], block t = rows t*128:(t+1)*128 of the shard
        xn = np.ascontiguousarray(
            shard.reshape(NB, 128, IN).transpose(1, 0, 2).reshape(128, NB * IN)
        )
        in_maps.append({"xt": xt, "xn": xn, "wf": wf})
    return in_maps


def kernel(x, base_weight, cheb_weight, degree=DEG, **_):
    assert int(degree) == DEG
    nc = build_nc()
    in_maps = make_in_maps(x, base_weight, cheb_weight)
    res = run_bass_kernel_spmd(nc, in_maps, list(range(NCORES)))
    return np.concatenate(
        [r["out"].astype(np.float32) for r in res.results], axis=0
    )
